# revision 20
# baseline (speedup 1.0000x reference)
"""Trainium2 Bass kernel for DeepInterestNetwork (DIN) — v4.

8 cores, data-parallel over batch; each core 512 rows = 8 tiles of G=64.
Host gathers embeddings and folds the query into per-row L1 weights; device
does the attention MLP, softmax, pooling, and head MLP.

Structure (per tile of 64 rows):
  - L1: h = relu(k @ W_r + cb) with per-row W_r = (Wk-Wd) + q_r*Wm and the
    bias folded in as a 65th contraction row (moving row = 64.0, stationary
    row = 256*cb).  Per row one fp8 matmul: stationary wrt[:, g, :] [65, 64],
    moving k8 [65, 200] contiguous -> psum 16384*h.  4 rows share one psum
    bank [128, 2, 256]; one pure-relu drain per bank, split 11 ACT / 5 DVE
    so the drain stream keeps pace with the PE.
  - L2: per pair p stationary w2p[:, p, :] (w2/64 at rows 2p, 2p+1), moving
    h [128, 200] f16 contiguous, 32 matmuls accumulate sc [64, 200] =
    256*scores.
  - softmax: one exp (scale 1/256) with accum_out giving the denominators;
    esn = es * recip(den) -> weights normalized BEFORE pooling.
  - pooling: esn striped to 128 partitions (2 gpsimd DMAs), DVE multiply
    tmp = stag * esn (2x mode), GPSIMD halving add (tmp2 = lo+hi), DVE
    reduce over the remaining 50 (1x — the only mode tensor_reduce has),
    one PE fold matmul (stationary red, moving p2m) -> 64*interest^T
    [64e, 64g] in psum, copied into itall by DVE (head weights are
    pre-divided by 64).
  - head MLP batched over all 512 rows at the end; sigmoid via exp + recip.

Pipeline lags (iteration i issues):  sync-ring k8x/wrt(i+3), act-ring
stag(i) | PE: L1(i) first (stays HAM-warm across the boundary), L2(i-1),
fold(i-3) | ACT: drains(i-1) blocks 13-15 at the head, itall-copy(i-4),
drains(i) blocks 0-12 inline with L1, exp(i-1) last | DVE: recip/esn(i-2),
mul+reduce(i-3) — nothing else, so the pooling chain never blocks a drain |
GPSIMD: esp stripes(i-2).
"""

import numpy as np
import sys

for p in ("/opt/trn_rl_repo", "/opt/trn_rl_repo/concourse"):
    if p not in sys.path:
        sys.path.insert(0, p)

VOCAB, E = 100000, 64
B, T = 4096, 200
NCORES = 8
BC = B // NCORES          # 512 rows per core
G = 64                    # batch rows per tile
NTILES_FULL = BC // G     # 8
NQ = 2                    # t-halves: partition p = 64*qt + g, t = 100*qt + r
RANKS = T // NQ           # 100
HR = RANKS // 2           # 50 after the halving add
NPAIR = G // 2            # 32 row pairs
NBLK = G // 4             # 16 psum blocks (4 rows each)
NDRA = 13                 # psum blocks drained by ACT (rest by DVE)
SCALE = 16384.0           # k*64 (fp8) x W*256 (fp8) = 16384*z

_nc_cache = {}


def build_nc(ntiles=NTILES_FULL):
    import concourse.bacc as bacc
    import concourse.mybir as mybir
    import concourse.tile as tile

    f32 = mybir.dt.float32
    f16 = mybir.dt.float16
    f8 = mybir.dt.float8e4
    AF = mybir.ActivationFunctionType
    AX = mybir.AxisListType
    ALU = mybir.AluOpType

    nc = bacc.Bacc("TRN2", target_bir_lowering=False, debug=False)

    k8x_d = nc.dram_tensor("k8x", [ntiles, E + 1, G, T], f8,
                           kind="ExternalInput")
    wrt_d = nc.dram_tensor("wrt", [ntiles, E + 1, G, E], f8,
                           kind="ExternalInput")
    stag_d = nc.dram_tensor("stag", [ntiles, 128, E, RANKS], f16,
                            kind="ExternalInput")
    w2p_d = nc.dram_tensor("w2p", [128, NPAIR, G], f16, kind="ExternalInput")
    p2m_d = nc.dram_tensor("p2m", [128, G], f16, kind="ExternalInput")
    dw1_d = nc.dram_tensor("dw1", [E, 128], f16, kind="ExternalInput")
    db1_d = nc.dram_tensor("db1", [128, 1], f32, kind="ExternalInput")
    dw2_d = nc.dram_tensor("dw2", [128, E], f16, kind="ExternalInput")
    db2_d = nc.dram_tensor("db2", [E, 1], f32, kind="ExternalInput")
    ow_d = nc.dram_tensor("ow", [E, 1], f16, kind="ExternalInput")
    obn_d = nc.dram_tensor("obn", [1, 1], f32, kind="ExternalInput")
    out_d = nc.dram_tensor("out", [1, ntiles * G], f32, kind="ExternalOutput")

    with tile.TileContext(nc) as tc:
        with tc.tile_pool(name="consts", bufs=1) as consts, \
             tc.tile_pool(name="kqp", bufs=4) as kq_pool, \
             tc.tile_pool(name="wrp", bufs=4) as wr_pool, \
             tc.tile_pool(name="stp", bufs=3) as st_pool, \
             tc.tile_pool(name="hp", bufs=2) as h_pool, \
             tc.tile_pool(name="tmpp", bufs=2) as tmp_pool, \
             tc.tile_pool(name="t2p", bufs=2) as t2_pool, \
             tc.tile_pool(name="smp", bufs=3) as sm_pool, \
             tc.tile_pool(name="espp", bufs=3) as esp_pool, \
             tc.tile_pool(name="redp", bufs=3) as red_pool, \
             tc.tile_pool(name="php", bufs=4, space="PSUM") as ph_pool, \
             tc.tile_pool(name="scp", bufs=2, space="PSUM") as sc_pool, \
             tc.tile_pool(name="pmp", bufs=2, space="PSUM") as pm_pool:

            def load_l1(ti):
                k8 = kq_pool.tile([E + 1, G, T], f8, tag="k8")
                nc.sync.dma_start(k8[:], k8x_d.ap()[ti])
                wr = wr_pool.tile([E + 1, G, E], f8, tag="wr")
                nc.sync.dma_start(wr[:], wrt_d.ap()[ti])
                return {"k8": k8, "wr": wr}

            def load_stag(ti):
                # act-ring (scalar queue) so bulk loads use both HW-DGE rings
                st = st_pool.tile([128, E, RANKS], f16, tag="st")
                nc.scalar.dma_start(st[:], stag_d.ap()[ti])
                return st

            loads = {}

            def load_l1_first(ti):
                wr = wr_pool.tile([E + 1, G, E], f8, tag="wr")
                nc.sync.dma_start(wr[:], wrt_d.ap()[ti])
                k8 = kq_pool.tile([E + 1, G, T], f8, tag="k8")
                nc.sync.dma_start(k8[:], k8x_d.ap()[ti])
                return {"k8": k8, "wr": wr}

            loads[0] = load_l1_first(0)
            loads[1] = load_l1(1)

            # ---- constants (after the first two tile loads) ----
            w2p = consts.tile([128, NPAIR, G], f16)
            nc.sync.dma_start(w2p[:], w2p_d.ap())
            p2m = consts.tile([128, G], f16)
            nc.sync.dma_start(p2m[:], p2m_d.ap())
            loads[2] = load_l1(2)
            dw1 = consts.tile([E, 128], f16)
            nc.sync.dma_start(dw1[:], dw1_d.ap())
            db1 = consts.tile([128, 1], f32)
            nc.sync.dma_start(db1[:], db1_d.ap())
            dw2 = consts.tile([128, E], f16)
            nc.sync.dma_start(dw2[:], dw2_d.ap())
            db2 = consts.tile([E, 1], f32)
            nc.sync.dma_start(db2[:], db2_d.ap())
            ow = consts.tile([E, 1], f16)
            nc.sync.dma_start(ow[:], ow_d.ap())
            obn = consts.tile([1, 1], f32)
            nc.sync.dma_start(obn[:], obn_d.ap())
            itall = consts.tile([E, ntiles * G], f16)

            stations = {}  # ti -> dict of live tiles per pipeline station

            for i in range(ntiles + 5):
                # ---- prefetch ----
                if 0 <= i < ntiles:
                    stations.setdefault(i, {})["st"] = load_stag(i)
                if 3 <= i + 3 < ntiles:
                    loads[i + 3] = load_l1(i + 3)

                # ---- ACT: previous tile's last drains, then old copy ----
                if 0 <= i - 1 < ntiles:
                    ent = stations[i - 1]
                    for m in range(NDRA, NBLK):
                        nc.scalar.activation(ent["hall"][:, m, :, :],
                                             ent["ph"][m][:, :, 0:T], AF.Relu)
                    ent.pop("ph")
                if 0 <= i - 4 < ntiles:
                    ent = stations.pop(i - 4)
                    nc.scalar.copy(
                        itall[:, (i - 4) * G : (i - 3) * G], ent["pit"][:]
                    )

                # ---- DVE: pooling mul (i-2) then reduce (i-3) ----
                if 0 <= i - 2 < ntiles:
                    ent = stations[i - 2]
                    tmp = tmp_pool.tile([128, E, RANKS], f16, tag="tmp")
                    nc.vector.tensor_mul(
                        tmp[:], ent["st"][:],
                        ent["esp"][:, None, :].broadcast_to([128, E, RANKS]),
                    )
                    # GPSIMD halving add feeds the 1x-only tensor_reduce
                    tmp2 = t2_pool.tile([128, E, HR], f16, tag="tmp2")
                    nc.gpsimd.tensor_add(tmp2[:], tmp[:, :, 0:HR],
                                         tmp[:, :, HR:RANKS])
                    ent["tmp2"] = tmp2
                if 0 <= i - 3 < ntiles:
                    ent = stations[i - 3]
                    red = red_pool.tile([128, E], f16, tag="red")
                    with nc.allow_low_precision("softmax weights sum to 1"):
                        nc.vector.tensor_reduce(red[:], ent["tmp2"][:],
                                                AX.X, ALU.add)
                    ent["red"] = red

                # ---- PE: L2 (i-1) at the head (drains trickle in) ----
                if 0 <= i - 1 < ntiles:
                    ent = stations[i - 1]
                    hall = ent["hall"]
                    sc = sc_pool.tile([G, 256], f32, tag="sc")
                    for m in range(NBLK):
                        for s_ in range(2):
                            pr = 2 * m + s_
                            nc.tensor.matmul(
                                sc[:, 0:T], w2p[:, pr, :], hall[:, m, s_, :],
                                start=(pr == 0), stop=(pr == NPAIR - 1),
                                skip_group_check=True,
                            )
                    es = sm_pool.tile([G, NQ, RANKS], f16, tag="es")
                    den = sm_pool.tile([G, 1], f32, tag="den")
                    es_v = es[:].rearrange("p q r -> p (q r)")
                    nc.scalar.activation(es_v, sc[:, 0:T], AF.Exp,
                                         scale=1.0 / 256, accum_out=den[:])
                    # DVE normalize (after this iteration's mul+reduce)
                    rd = sm_pool.tile([G, 1], f32, tag="rd")
                    nc.vector.reciprocal(rd[:], den[:])
                    esn = sm_pool.tile([G, NQ, RANKS], f16, tag="esn")
                    nc.vector.tensor_scalar_mul(esn[:], es[:], rd[:])
                    esp = esp_pool.tile([128, RANKS], f16, tag="esp")
                    nc.gpsimd.dma_start(esp[0:G, :], esn[:, 0, :])
                    nc.gpsimd.dma_start(esp[G : 2 * G, :], esn[:, 1, :])
                    ent["esp"] = esp

                # ---- PE: L1 (i), ACT drains blocks 0..NDRA-1 ----
                if i < ntiles:
                    ld = loads.pop(i)
                    k8, wr = ld["k8"], ld["wr"]
                    hall = h_pool.tile([128, NBLK, 2, T], f16, tag="hall")
                    phs = {}
                    for m in range(NBLK):
                        ph = ph_pool.tile([128, 2, 256], f32, tag="ph")
                        for s_ in range(2):
                            for par in range(2):
                                g = 4 * m + 2 * s_ + par
                                nc.tensor.matmul(
                                    ph[64 * par : 64 * par + 64, s_, 0:T],
                                    wr[:, g, :], k8[:, g, :],
                                )
                        if m < NDRA:
                            nc.scalar.activation(hall[:, m, :, :],
                                                 ph[:, :, 0:T], AF.Relu)
                        else:
                            phs[m] = ph
                    stations[i]["hall"] = hall
                    stations[i]["ph"] = phs

                # ---- PE: fold (i-3) at the end of the PE program ----
                if 0 <= i - 3 < ntiles:
                    ent = stations[i - 3]
                    pit = pm_pool.tile([E, G], f32, tag="pm")
                    nc.tensor.matmul(pit[:], ent["red"][:], p2m[:])
                    ent["pit"] = pit

            # ---- epilogue: batched head MLP over all 512 rows ----
            pd1 = ph_pool.tile([128, ntiles * G], f32, tag="ph")
            nc.tensor.matmul(pd1[:], dw1[:], itall[:])
            d1h = sm_pool.tile([128, ntiles * G], f16, tag="d1h")
            nc.scalar.activation(d1h[:], pd1[:], AF.Relu, bias=db1[:])
            pd2 = ph_pool.tile([E, ntiles * G], f32, tag="ph")
            nc.tensor.matmul(pd2[:], dw2[:], d1h[:])
            d2h = sm_pool.tile([E, ntiles * G], f16, tag="d2h")
            nc.scalar.activation(d2h[:], pd2[:], AF.Relu, bias=db2[:])
            po = ph_pool.tile([1, ntiles * G], f32, tag="ph")
            nc.tensor.matmul(po[:], ow[:], d2h[:])
            # sigmoid(x+ob) = 1/(1+exp(-x-ob))
            en = sm_pool.tile([1, ntiles * G], f32, tag="en")
            nc.scalar.activation(en[:], po[:], AF.Exp, scale=-1.0, bias=obn[:])
            sp1 = sm_pool.tile([1, ntiles * G], f32, tag="sp1")
            nc.vector.tensor_scalar_add(sp1[:], en[:], 1.0)
            outall = sm_pool.tile([1, ntiles * G], f32, tag="outall")
            nc.vector.reciprocal(outall[:], sp1[:])
            nc.sync.dma_start(out_d.ap(), outall[:])

    nc.compile()
    return nc


def marshal_inputs(query, keys, emb, att_w1, att_b1, att_w2, att_b2,
                   deep_w1, deep_b1, deep_w2, deep_b2, out_w, out_b,
                   ntiles=NTILES_FULL):
    import concourse.mybir as mybir
    f8np = mybir.dt.np(mybir.dt.float8e4)

    query = np.asarray(query).astype(np.int64)
    keys = np.asarray(keys).astype(np.int64)
    emb = np.asarray(emb, dtype=np.float32)
    a1 = np.asarray(att_w1, dtype=np.float32)
    Wq, Wk, Wd, Wm = a1[0:64], a1[64:128], a1[128:192], a1[192:256]
    Wkd = Wk - Wd
    Wqd = Wq + Wd
    b1 = np.asarray(att_b1, np.float32)
    w2 = np.asarray(att_w2, np.float32)[:, 0]

    # constants (shared across cores)
    w2p = np.zeros((128, NPAIR, G), np.float16)
    for pr in range(NPAIR):
        w2p[0:64, pr, 2 * pr] = (w2 / 64.0).astype(np.float16)
        w2p[64:128, pr, 2 * pr + 1] = (w2 / 64.0).astype(np.float16)
    p2m = (np.arange(128)[:, None] % G == np.arange(G)[None, :]).astype(np.float16)
    # stag carries 64*k (keeps f16 tmp products out of the subnormal range);
    # divide deep_w1 by 64 to compensate.
    dw1 = (np.asarray(deep_w1, np.float32) / 64.0).astype(np.float16)
    db1 = np.asarray(deep_b1, np.float32).reshape(128, 1)
    dw2 = np.asarray(deep_w2, np.float32).astype(np.float16)
    db2 = np.asarray(deep_b2, np.float32).reshape(64, 1)
    ow = np.asarray(out_w, np.float32).astype(np.float16)
    obn = -np.asarray(out_b, np.float32).reshape(1, 1)

    in_maps = []
    for c in range(NCORES):
        rows = slice(c * BC, c * BC + ntiles * G)
        k32 = emb[keys[rows]]                      # [512, 200, 64] f32
        q32 = emb[query[rows]]                     # [512, 64] f32

        k32r = k32.reshape(ntiles, G, T, E)
        # k8x [nt, 65e, g, t]: 64*k, bias row = 64.0
        k8x = np.empty((ntiles, E + 1, G, T), np.float32)
        k8x[:, 0:E] = k32r.transpose(0, 3, 1, 2) * 64.0
        k8x[:, E] = 64.0
        k8x = k8x.astype(f8np)

        # wrt [nt, 65e, g, h]: 256*((Wk-Wd) + q_g*Wm); bias row 256*cb
        Wr = (Wkd[None, :, :] + q32[:, :, None] * Wm[None, :, :]) * 256.0
        cb = (q32 @ Wqd + b1) * 256.0              # [512, 64]
        wrt = np.empty((ntiles, E + 1, G, E), np.float32)
        wrt[:, 0:E] = Wr.reshape(ntiles, G, E, E).transpose(0, 2, 1, 3)
        wrt[:, E] = cb.reshape(ntiles, G, E)
        wrt = wrt.astype(f8np)

        # stag [nt, 128=(qt,g), e, r]: 64*k  (t = 100*qt + r)
        stag = np.ascontiguousarray(
            (k32r * 64.0).reshape(ntiles, G, NQ, RANKS, E)
            .transpose(0, 2, 1, 4, 3)
            .reshape(ntiles, 128, E, RANKS)
        ).astype(np.float16)

        in_maps.append({
            "k8x": k8x, "wrt": wrt, "stag": stag,
            "w2p": w2p, "p2m": p2m,
            "dw1": dw1, "db1": db1, "dw2": dw2, "db2": db2,
            "ow": ow, "obn": obn,
        })
    return in_maps


def kernel(**inputs) -> np.ndarray:
    from concourse.bass_utils import run_bass_kernel_spmd

    if "full" not in _nc_cache:
        _nc_cache["full"] = build_nc(NTILES_FULL)
    nc = _nc_cache["full"]
    in_maps = marshal_inputs(**inputs)
    res = run_bass_kernel_spmd(nc, in_maps, core_ids=list(range(NCORES)))
    outs = [np.asarray(res.results[c]["out"]).reshape(-1) for c in range(NCORES)]
    return np.concatenate(outs).reshape(B, 1).astype(np.float32)


if __name__ == "__main__":
    sys.path.insert(0, "/root/problem")
    import reference
    inputs = {k: np.asarray(v) for k, v in reference.setup_inputs().items()}
    expected = np.asarray(reference.reference(**inputs))
    actual = kernel(**inputs)
    err = np.abs(actual - expected).max() / (np.abs(expected).max() + 1e-12)
    print("Relative error:", err)
